# revision 17
# baseline (speedup 1.0000x reference)
"""Trainium2 Bass kernel for gnn_message_passing (nn_BFR_28089086116615).

v2 design (transposed-primary):
- Receiver axis i sharded (G=4096 -> 8 cores x 512). Edge matrices are
  host-transposed and gated in bf16: wT[j, i].
- Phase 1 computes h0^T = elu(W^T x) in a column-folded layout (4 row-groups
  of 32 features x 2048 cols) with 16 big N=512 bf16 matmuls, then 16 PE
  transposes ([128,128] blocks) rebuild the natural [gene-partition | 1|h]
  groups for the contraction lhsT. Per-core gene permutation (local genes
  first) keeps the local receiver slice at a fixed, core-independent offset.
- All small matmuls run in bf16 (1 cyc/row on PE vs 4 for fp32).
- Partition broadcasts (sdb row, rowsum, BN scale/shift) run on GpSimd
  instead of PE+DVE.
- BatchNorm is per-gene -> fully local, computed in row layout only.
- sigma^T is produced chunk-wise by ACT (sigmoid, per-partition bias =
  ssrc[j-chunk]), gated on DVE in bf16, contracted on PE with stationary
  [1|h] groups so the receiver rowsum lands in psum row 0.
- ACT work is batched by table set (sigmoid vs exp) to minimize the 1.3us
  activation table reloads.
- One AllGather of the normalized h (natural [1|h] groups) between blocks;
  scattered to SBUF with a single strided DMA.
"""
import sys
sys.path.insert(0, "/opt/trn_rl_repo")
import numpy as np
import ml_dtypes

import concourse.bass as bass
import concourse.bacc as bacc
import concourse.mybir as mybir
import concourse.tile as tile
from concourse import masks
from concourse.bass_utils import run_bass_kernel_spmd

NC = 8
B, G, NI, H, NO = 2, 4096, 8, 32, 32
GL = G // NC              # 512 local receivers per core
LCH = GL // 128           # 4 local chunks
NCH = G // 128            # 32 global j-chunks
W1 = H + 1                # group width: [1 | h]
ALPHA, BETA, BN_EPS = 0.005, 5e-5, 1e-5

F32 = mybir.dt.float32
F32R = mybir.dt.float32r
BF16 = mybir.dt.bfloat16
AF = mybir.ActivationFunctionType
ALU = mybir.AluOpType
XY = mybir.AxisListType.XY
AX = mybir.AxisListType.X

# pack_b column layout (bf16)
PK_WREP1 = 0                      # [128, NCH*W1] wrep1e (b_e1 in slot 0)
PK_WREP2 = NCH * W1               # [128, NCH*W1]
PK_BASE = 2 * NCH * W1            # 2112
PK_WAUG = PK_BASE                 # [9, 32]
PK_WE1D = PK_BASE + 32            # [32, 1]
PK_WE2D = PK_BASE + 33            # [32, 1]
PK_W8 = PK_BASE + 34              # 8 blocks of 32: Wn1a Wn1b Wm1a Wm1b Wn2a Wn2b Wm2a Wm2b
PK_W = PK_W8 + 8 * 32             # total width 2402
# pack_f column layout (fp32): 8 weight blocks, then bn_g/bn_b rows
PF_W8 = 0
PF_BNG = 256
PF_BNB = PF_BNG + GL
PF_ONER = PF_BNB + GL             # [1, GL] ones row
PF_ONEC = PF_ONER + GL            # [33, 1] ones column
PF_W = PF_ONEC + 1

_CACHE = {}


def build_program():
    nc = bacc.Bacc("TRN2", target_bir_lowering=False, debug=False,
                   enable_asserts=False, num_devices=NC)

    def din(name, shape, dt):
        return nc.dram_tensor(name, shape, dt, kind="ExternalInput").ap()

    xT_b = din("xT_b", [NI + 1, B * G], BF16)
    w1T = din("w1T", [G, GL], BF16)
    w2T = din("w2T", [G, GL], BF16)
    pack_b = din("pack_b", [128, PK_W], BF16)
    pack_f = din("pack_f", [33, PF_W], F32R)

    out = nc.dram_tensor("out", [B * GL, NO], F32, kind="ExternalOutput").ap()
    out_r = out.rearrange("(b l p) f -> p b l f", b=B, l=LCH, p=128)

    with tile.TileContext(nc) as tc:
        with (
            tc.tile_pool(name="cp", bufs=1) as cp,
            tc.tile_pool(name="bp", bufs=1) as bp,
            tc.tile_pool(name="wp", bufs=1) as wp,
            tc.tile_pool(name="sp", bufs=2) as sp,
            tc.tile_pool(name="pp", bufs=1, space="PSUM") as pp,
            tc.tile_pool(name="dp", bufs=1, space="DRAM") as dp,
        ):
            # ---------------- input DMAs (small gating ones first) --------
            pack_sb = cp.tile([128, PK_W], BF16, name="pack_sb", tag="pack_sb")
            nc.sync.dma_start(pack_sb[:, PK_BASE:PK_W],
                              pack_b[:, PK_BASE:PK_W])
            xq = cp.tile([NI + 1, B * G], BF16, name="xq", tag="xq")
            for s in range(4):
                nc.sync.dma_start(xq[:, s * 2048:(s + 1) * 2048],
                                  xT_b[:, s * 2048:(s + 1) * 2048])
            pack_f_sb = cp.tile([33, PF_W], F32R, name="pack_f_sb",
                                tag="pack_f_sb")
            nc.sync.dma_start(pack_f_sb[:], pack_f[:])
            nc.sync.dma_start(pack_sb[:, 0:PK_BASE], pack_b[:, 0:PK_BASE])
            w1T_sb = bp.tile([128, NCH * GL], BF16, name="w1T_sb", tag="w1T_sb")
            w2T_sb = bp.tile([128, NCH * GL], BF16, name="w2T_sb", tag="w2T_sb")
            w1T_r = w1T.rearrange("(k p) i -> p k i", p=128)
            w2T_r = w2T.rearrange("(k p) i -> p k i", p=128)
            for kq in range(4):
                nc.sync.dma_start(
                    w1T_sb[:, kq * 8 * GL:(kq + 1) * 8 * GL],
                    w1T_r[:, kq * 8:(kq + 1) * 8])

            # views into the const pack
            wrep1_v = pack_sb[:, PK_WREP1:PK_WREP1 + NCH * W1].rearrange(
                "p (k e) -> p k e", e=W1)
            wrep2_v = pack_sb[:, PK_WREP2:PK_WREP2 + NCH * W1].rearrange(
                "p (k e) -> p k e", e=W1)
            W_aug = pack_sb[0:NI + 1, PK_WAUG:PK_WAUG + 32]
            We1_d = pack_sb[0:H, PK_WE1D:PK_WE1D + 1]
            We2_d = pack_sb[0:H, PK_WE2D:PK_WE2D + 1]

            def wblk(i, p):
                return pack_f_sb[0:p, PF_W8 + i * 32:PF_W8 + (i + 1) * 32]
            Wn1a, Wn1b = wblk(0, 33), wblk(1, 33)
            Wm1a, Wm1b = wblk(2, 33), wblk(3, 32)
            Wn2a, Wn2b = wblk(4, 33), wblk(5, 33)
            Wm2a, Wm2b = wblk(6, 33), wblk(7, 32)
            Wm1b_h = pack_sb[0:32, PK_W8 + 3 * 32:PK_W8 + 4 * 32]
            bn_g = pack_f_sb[0:1, PF_BNG:PF_BNG + GL].bitcast(F32)
            bn_b = pack_f_sb[0:1, PF_BNB:PF_BNB + GL].bitcast(F32)

            # ---------------- constants / identities ----------------------
            onesk_f = pack_f_sb[0:H, PF_ONEC:PF_ONEC + 1]
            oner_f = pack_f_sb[0:1, PF_ONER:PF_ONER + GL]
            id_bf = cp.tile([128, 128], BF16, name="id_bf", tag="id_bf")
            masks.make_identity(nc, id_bf[:])
            id_f = cp.tile([32, 32], F32, name="id_f", tag="id_f")
            masks.make_identity(nc, id_f[:])

            # ---------------- big resident tensors ------------------------
            h0T = bp.tile([128, 2048], BF16, name="h0T", tag="h0T")
            h0n = bp.tile([128, B * NCH * W1], BF16, name="h0n", tag="h0n")
            nc.vector.memset(h0n[:], 1.0)
            ghat = bp.tile([128, B * NCH * W1], BF16, name="ghat", tag="ghat")
            h1T = bp.tile([H, B * GL], F32R, name="h1T", tag="h1T")
            hbnT = bp.tile([H, B * GL], F32R, name="hbnT", tag="hbnT")
            hbn_b = bp.tile([H, B * GL], BF16, name="hbn_b", tag="hbn_b")
            hbn_n = wp.tile([128, B * LCH * W1], BF16, name="hbn_n",
                            tag="hbn_n")
            nc.vector.memset(hbn_n[:], 1.0)

            gins = [dp.tile([128, LCH * W1 + LCH], BF16, name=f"gin{i}",
                            tag=f"gin{i}") for i in range(B)]
            gouts = [dp.tile([NC * 128, LCH * W1 + LCH], BF16,
                             addr_space="Shared", name=f"gout{i}",
                             tag=f"gout{i}") for i in range(B)]

            hdTs, nodess = [], []
            for i in range(2):
                hdTx = wp.tile([W1, GL], F32R, name=f"hdT{i}", tag=f"hdT{i}")
                nc.vector.tensor_copy(hdTx[H:W1, :], oner_f)
                hdTs.append(hdTx)
                nodesx = wp.tile([W1, GL], F32R, name=f"nodes{i}",
                                 tag=f"nodes{i}")
                nc.vector.tensor_copy(nodesx[H:W1, :], oner_f)
                nodess.append(nodesx)

            def elu(z_psum, out_ap, p, f):
                t = wp.tile([128, 512], BF16, name="elu_t", tag="elu_t",
                            bufs=3)[0:p, 0:f]
                nc.vector.tensor_scalar_min(t, z_psum, 0.0)
                nc.scalar.activation(t, t, AF.Exp)
                nc.vector.tensor_scalar_add(t, t, -1.0)
                nc.vector.tensor_tensor(out_ap, z_psum, t, op=ALU.max)

            # ---------------- phase 1: h0T fold + natural -----------------
            # fold[32a+f, 512s+j] = h0T[f, (4s+a)*512+j]
            for s in range(4):
                ps = pp.tile([128, 512], F32, name="ps_ph0", tag="ph0", bufs=2)
                for a in range(4):
                    q = 4 * s + a
                    nc.tensor.matmul(ps[32 * a:32 * a + 32, :], W_aug,
                                     xq[:, q * 512:(q + 1) * 512],
                                     start=True, stop=True,
                                     tile_position=(0, 32 * a))
                elu(ps[:], h0T[:, s * 512:(s + 1) * 512], 128, 512)
            # natural layout via PE transposes: group m = 16u + t1 + 4a
            h0n_g = h0n.rearrange("p (u a t e) -> p u a t e", u=4, a=4, t=4)
            for u in range(4):
                trp = pp.tile([128, 512], BF16, name="ps_tr", tag="tr", bufs=1)
                for t1 in range(4):
                    nc.tensor.transpose(
                        trp[:, t1 * 128:(t1 + 1) * 128],
                        h0T[:, (4 * u + t1) * 128:(4 * u + t1 + 1) * 128],
                        id_bf[:])
                trp_v = trp.rearrange("p (t a e) -> p t a e", t=4, a=4)
                nc.vector.tensor_copy(
                    h0n_g[:, u, :, :, 1:W1].transpose([0, 2, 1, 3]), trp_v)

            # local receiver features: h0l(b) = fold[0:32, 1024b : 1024b+512]
            def h0l(b):
                return h0T[0:H, 1024 * b:1024 * b + 512]

            h0n_v = h0n.rearrange("p (g e) -> p g e", e=W1)
            ghat_v = ghat.rearrange("p (g e) -> p g e", e=W1)

            # issue w2T loads behind w1T
            for kq in range(4):
                nc.sync.dma_start(
                    w2T_sb[:, kq * 8 * GL:(kq + 1) * 8 * GL],
                    w2T_r[:, kq * 8:(kq + 1) * 8])

            # ---------------- shared mp-block pieces ----------------------
            def ssrc_calc(blk, nat_v, wrep_v):
                ssrc = wp.tile([128, B * NCH], F32, name=f"ssrc{blk}",
                               tag=f"ssrc{blk}")
                for b in range(B):
                    ssx = wp.tile([128, NCH * W1], BF16, name="ssx", tag="ssx",
                                  bufs=2)
                    ssx_v = ssx.rearrange("p (k e) -> p k e", e=W1)
                    nc.vector.tensor_tensor(
                        ssx_v, nat_v[:, b * NCH:(b + 1) * NCH, :], wrep_v,
                        op=ALU.mult)
                    nc.vector.reduce_sum(ssrc[:, b * NCH:(b + 1) * NCH],
                                         ssx_v, axis=AX)
                return ssrc

            def sdb_calc(blk, We_d, hTl_of):
                sd_row = wp.tile([1, B * GL], BF16, name=f"sd{blk}",
                                 tag=f"sd{blk}")
                for b in range(B):
                    ps_d = pp.tile([1, 512], F32, name="ps_d", tag="sm",
                                   bufs=2)
                    nc.tensor.matmul(ps_d[:], We_d, hTl_of(b),
                                     start=True, stop=True)
                    nc.vector.tensor_copy(sd_row[:, b * GL:(b + 1) * GL],
                                          ps_d[:])
                sdb = wp.tile([128, B * GL], BF16, name=f"sdb{blk}",
                              tag=f"sdb{blk}")
                nc.gpsimd.partition_broadcast(sdb[:], sd_row[:])
                return sdb

            def contraction(b, wT_sb, sdb, ssrc, nat_v):
                ps_acc = pp.tile([W1, GL], F32, name="ps_acc", tag="acc",
                                 bufs=2)
                for qq in range(4):
                    sig = sp.tile([128, 8 * GL], BF16, name="sig", tag="sig",
                                  bufs=2)
                    for k8 in range(8):
                        k = qq * 8 + k8
                        nc.scalar.activation(
                            sig[:, k8 * GL:(k8 + 1) * GL],
                            sdb[:, b * GL:(b + 1) * GL], AF.Sigmoid,
                            bias=ssrc[:, b * NCH + k:b * NCH + k + 1])
                    for hh in range(2):
                        sl = slice(hh * 4 * GL, (hh + 1) * 4 * GL)
                        wsl = slice((qq * 8 + hh * 4) * GL,
                                    (qq * 8 + hh * 4 + 4) * GL)
                        nc.vector.tensor_tensor(sig[:, sl], sig[:, sl],
                                                wT_sb[:, wsl], op=ALU.mult)
                    for k8 in range(8):
                        k = qq * 8 + k8
                        nc.tensor.matmul(
                            ps_acc[:], nat_v[:, b * NCH + k, :],
                            sig[:, k8 * GL:(k8 + 1) * GL],
                            start=(k == 0), stop=(k == NCH - 1))
                return ps_acc

            def post_acc(b, ps_acc, hTl_b, hTl_mrg, Wna, Wnb, Wma, Wmb,
                         mrg_out, mrg_p):
                rfull = wp.tile([W1, GL], F32R, name="rfull", tag="rfull",
                                bufs=2)
                nc.vector.tensor_copy(rfull[:], ps_acc[:])
                rb = wp.tile([H, GL], F32R, name="rb", tag="rb", bufs=2)
                nc.gpsimd.partition_broadcast(rb[:], rfull[0:1, :])
                hdT = hdTs[b]
                nc.vector.tensor_tensor(hdT[0:H, :], hTl_b, rb.bitcast(F32),
                                        op=ALU.mult)
                ps_n = pp.tile([H, GL], F32, name="ps_n", tag="sm", bufs=2)
                nc.tensor.matmul(ps_n[:], Wna, rfull[:],
                                 start=True, stop=False)
                nc.tensor.matmul(ps_n[:], Wnb, hdT[:],
                                 start=False, stop=True)
                nodes = nodess[b]
                elu(ps_n[:], nodes[0:H, :], H, GL)
                ps_m = pp.tile([H, GL], F32, name="ps_m", tag="sm", bufs=2)
                nc.tensor.matmul(ps_m[:], Wma, nodes[:],
                                 start=True, stop=False)
                hT_m, Wmb_m = hTl_mrg
                nc.tensor.matmul(ps_m[:], Wmb_m, hT_m, start=False, stop=True)
                elu(ps_m[:], mrg_out, mrg_p, GL)

            # ---------------- block 1 -------------------------------------
            ssrc1 = ssrc_calc(1, h0n_v, wrep1_v)
            sdb1 = sdb_calc(1, We1_d, h0l)
            accs = [contraction(b, w1T_sb, sdb1, ssrc1, h0n_v)
                    for b in range(B)]
            for b in range(B):
                post_acc(b, accs[b], h0l(b), (h0l(b), Wm1b_h),
                         Wn1a, Wn1b, Wm1a, Wm1b,
                         h1T[:, b * GL:(b + 1) * GL], H)

            # ---------------- BatchNorm (row layout, fully local) ---------
            sqT = wp.tile([H, B * GL], F32R, name="sqT", tag="sqT")
            nc.vector.tensor_tensor(sqT[:], h1T.bitcast(F32), h1T.bitcast(F32),
                                    op=ALU.mult)
            ps_r0 = pp.tile([1, GL], F32, name="ps_r0", tag="sm", bufs=2)
            for b in range(B):
                nc.tensor.matmul(ps_r0[:], onesk_f[:],
                                 h1T[:, b * GL:(b + 1) * GL],
                                 start=(b == 0), stop=(b == B - 1))
            ps_r1 = pp.tile([1, GL], F32, name="ps_r1", tag="sm", bufs=2)
            for b in range(B):
                nc.tensor.matmul(ps_r1[:], onesk_f[:],
                                 sqT[:, b * GL:(b + 1) * GL],
                                 start=(b == 0), stop=(b == B - 1))
            rowb = wp.tile([1, 4 * GL], F32, name="rowb", tag="rowb")
            mu_r, var_r = rowb[:, 0:GL], rowb[:, GL:2 * GL]
            scl_r, shf_r = rowb[:, 2 * GL:3 * GL], rowb[:, 3 * GL:4 * GL]
            nc.vector.tensor_scalar_mul(mu_r, ps_r0[:], 1.0 / (B * NO))
            nc.vector.tensor_scalar_mul(var_r, ps_r1[:], 1.0 / (B * NO))
            nc.vector.tensor_tensor(scl_r, mu_r, mu_r, op=ALU.mult)
            nc.vector.tensor_tensor(var_r, var_r, scl_r, op=ALU.subtract)
            nc.vector.tensor_scalar_add(scl_r, var_r, BN_EPS)
            nc.scalar.activation(scl_r, scl_r, AF.Ln)
            nc.scalar.activation(scl_r, scl_r, AF.Exp, scale=-0.5)
            nc.vector.tensor_tensor(scl_r, scl_r, bn_g, op=ALU.mult)
            nc.vector.tensor_tensor(shf_r, mu_r, scl_r, op=ALU.mult)
            nc.vector.tensor_tensor(shf_r, bn_b, shf_r, op=ALU.subtract)
            ssb = wp.tile([H, 2 * GL], F32, name="ssb", tag="ssb")
            nc.gpsimd.partition_broadcast(ssb[:], rowb[:, 2 * GL:4 * GL])
            for b in range(B):
                sl = slice(b * GL, (b + 1) * GL)
                nc.vector.tensor_tensor(hbnT[:, sl],
                                        h1T[:, sl].bitcast(F32), ssb[:, 0:GL],
                                        op=ALU.mult)
                nc.vector.tensor_tensor(hbnT[:, sl], hbnT[:, sl].bitcast(F32),
                                        ssb[:, GL:2 * GL], op=ALU.add)
            nc.vector.tensor_copy(hbn_b[:], hbnT.bitcast(F32))

            # block-2 sdb can go before the gather (local only)
            sdb2 = sdb_calc(2, We2_d,
                            lambda b: hbn_b[:, b * GL:(b + 1) * GL])

            # ---------------- per-batch gathers of normalized h -----------
            # payload per batch: [4 x (1|h) groups | 4 ssrc2 scalars]
            for b in range(B):
                hbnb = wp.tile([128, LCH * W1 + LCH], BF16, name="hbnb",
                               tag="hbnb", bufs=2)
                nc.vector.memset(hbnb[:], 1.0)
                trh = pp.tile([128, 512], BF16, name="ps_trh", tag="tr",
                              bufs=1)
                for l in range(LCH):
                    j = b * LCH + l
                    nc.tensor.transpose(trh[:, l * 32:(l + 1) * 32],
                                        hbn_b[:, j * 128:(j + 1) * 128],
                                        id_bf[0:H, 0:H])
                hbnb_g = hbnb[:, 0:LCH * W1].rearrange("p (g e) -> p g e",
                                                       e=W1)
                trh_v = trh[:, 0:LCH * H].rearrange("p (g e) -> p g e", e=H)
                nc.vector.tensor_copy(hbnb_g[:, :, 1:W1], trh_v)
                ssx2 = wp.tile([128, LCH * W1], BF16, name="ssx2", tag="ssx2",
                               bufs=2)
                ssx2_v = ssx2.rearrange("p (k e) -> p k e", e=W1)
                nc.vector.tensor_tensor(ssx2_v, hbnb_g, wrep2_v[:, 0:LCH, :],
                                        op=ALU.mult)
                s2f = wp.tile([128, LCH], F32, name="s2f", tag="s2f", bufs=2)
                nc.vector.reduce_sum(s2f[:], ssx2_v, axis=AX)
                nc.vector.tensor_copy(hbnb[:, LCH * W1:LCH * W1 + LCH],
                                      s2f[:])
                nc.sync.dma_start(gins[b][:], hbnb[:])
                nc.gpsimd.collective_compute(
                    "AllGather", ALU.bypass, replica_groups=[list(range(NC))],
                    ins=[gins[b].opt()], outs=[gouts[b].opt()])

            # ---------------- block 2 -------------------------------------
            ssrc2 = wp.tile([128, B * NCH], F32, name="ssrc2", tag="ssrc2")
            ssrc2b = wp.tile([128, B * NCH], BF16, name="ssrc2b",
                             tag="ssrc2b")
            for b in range(B):
                gr = gouts[b][:, 0:LCH * W1].rearrange("(c p) x -> p c x",
                                                       p=128)
                nc.sync.dma_start(
                    ghat[:, b * NCH * W1:(b + 1) * NCH * W1].rearrange(
                        "p (c x) -> p c x", c=NC), gr)
                sr = gouts[b][:, LCH * W1:LCH * W1 + LCH].rearrange(
                    "(c p) l -> p c l", p=128)
                nc.sync.dma_start(
                    ssrc2b[:, b * NCH:(b + 1) * NCH].rearrange(
                        "p (c l) -> p c l", c=NC), sr)
                nc.vector.tensor_copy(ssrc2[:, b * NCH:(b + 1) * NCH],
                                      ssrc2b[:, b * NCH:(b + 1) * NCH])
            out2T = wp.tile([H, B * GL], F32, name="out2T", tag="out2T")
            accs2 = [contraction(b, w2T_sb, sdb2, ssrc2, ghat_v)
                     for b in range(B)]
            for b in range(B):
                sl2 = slice(b * GL, (b + 1) * GL)
                post_acc(b, accs2[b], hbnT[:, sl2].bitcast(F32),
                         (hbnT[:, sl2], Wm2b),
                         Wn2a, Wn2b, Wm2a, Wm2b,
                         out2T[:, sl2], H)

            # transpose to natural and store
            tro = pp.tile([128, B * LCH * NO], F32, name="ps_tro", tag="trf",
                          bufs=1)
            for j in range(B * LCH):
                nc.tensor.transpose(tro[:, j * 32:(j + 1) * 32],
                                    out2T[:, j * 128:(j + 1) * 128],
                                    id_f[:])
            out_n = wp.tile([128, B * LCH * NO], F32, name="out_n",
                            tag="out_n")
            nc.vector.tensor_copy(out_n[:], tro[:])
            nc.sync.dma_start(
                out_r, out_n.rearrange("p (b l f) -> p b l f", b=B, l=LCH))

    nc.compile()
    return nc


def _prep_inputs(x, edges1, edges2, W_infer, b_infer, W_e1, b_e1, W_e2, b_e2,
                 W_n1, b_n1, W_n2, b_n2, W_m1, b_m1, W_m2, b_m2,
                 bn_gamma, bn_beta):
    f32 = np.float32
    bf16 = ml_dtypes.bfloat16
    x = np.asarray(x, f32)
    w1 = (ALPHA + (1.0 - ALPHA) * np.asarray(edges1, f32)).astype(bf16)
    w2 = (BETA + (1.0 - BETA) * np.asarray(edges2, f32)).astype(bf16)

    def wrepe(W_e, b_e):
        row = np.concatenate([[np.asarray(b_e, f32)[0]],
                              np.asarray(W_e, f32)[:H, 0]])
        return np.tile(row, (128, NCH)).astype(bf16)

    z = np.zeros((1, NO), f32)

    def stk(Wpart, brow):
        return np.concatenate([np.asarray(Wpart, f32), brow], 0)

    pack = np.zeros((128, PK_W), f32)
    pack[:, PK_WREP1:PK_WREP1 + NCH * W1] = wrepe(W_e1, b_e1)
    pack[:, PK_WREP2:PK_WREP2 + NCH * W1] = wrepe(W_e2, b_e2)
    pack[0:NI + 1, PK_WAUG:PK_WAUG + 32] = np.concatenate(
        [np.asarray(W_infer, f32), np.asarray(b_infer, f32)[None, :]], 0)
    pack[0:H, PK_WE1D] = np.asarray(W_e1, f32)[H:, 0]
    pack[0:H, PK_WE2D] = np.asarray(W_e2, f32)[H:, 0]
    blks = [
        (np.concatenate([z, np.asarray(W_n1, f32)[:H]], 0), 33),
        (stk(np.asarray(W_n1, f32)[H:], np.asarray(b_n1, f32)[None, :]), 33),
        (stk(np.asarray(W_m1, f32)[:H], np.asarray(b_m1, f32)[None, :]), 33),
        (np.asarray(W_m1, f32)[H:], 32),
        (np.concatenate([z, np.asarray(W_n2, f32)[:H]], 0), 33),
        (stk(np.asarray(W_n2, f32)[H:], np.asarray(b_n2, f32)[None, :]), 33),
        (stk(np.asarray(W_m2, f32)[:H], np.asarray(b_m2, f32)[None, :]), 33),
        (np.asarray(W_m2, f32)[H:], 32),
    ]
    packf = np.zeros((33, PF_W), f32)
    for i, (w, p) in enumerate(blks):
        pack[0:p, PK_W8 + i * 32:PK_W8 + (i + 1) * 32] = w
        packf[0:p, PF_W8 + i * 32:PF_W8 + (i + 1) * 32] = w
    packf[0, PF_ONER:PF_ONER + GL] = 1.0
    packf[:, PF_ONEC] = 1.0
    pack = pack.astype(bf16)

    in_maps = []
    for c in range(NC):
        sl = slice(c * GL, (c + 1) * GL)
        # per-core gene permutation for block 1: local genes first
        perm = np.concatenate([np.arange(c * GL, (c + 1) * GL),
                               np.arange(0, c * GL),
                               np.arange((c + 1) * GL, G)])
        xp = x[:, perm, :]                       # [B, G, NI] permuted
        xT = xp.transpose(2, 0, 1).reshape(NI, B * G)
        xT_b = np.concatenate([xT, np.ones((1, B * G), f32)], 0).astype(bf16)
        pf = packf.copy()
        pf[0, PF_BNG:PF_BNG + GL] = np.asarray(bn_gamma, f32)[sl]
        pf[0, PF_BNB:PF_BNB + GL] = np.asarray(bn_beta, f32)[sl]
        m = dict(
            xT_b=xT_b,
            w1T=np.ascontiguousarray(
                np.asarray(w1)[sl, :][:, perm].T),
            w2T=np.ascontiguousarray(np.asarray(w2)[sl, :].T),
            pack_b=pack,
            pack_f=pf,
        )
        in_maps.append(m)
    return in_maps


def kernel(**inputs):
    if "nc" not in _CACHE:
        _CACHE["nc"] = build_program()
    nc = _CACHE["nc"]
    in_maps = _prep_inputs(**inputs)
    res = run_bass_kernel_spmd(nc, in_maps, list(range(NC)))
    parts = [res.results[c]["out"].reshape(B, GL, NO) for c in range(NC)]
    return np.concatenate(parts, axis=1).astype(np.float32)


# revision 21
# speedup vs baseline: 1.0083x; 1.0083x over previous
"""Trainium2 Bass kernel for gnn_message_passing (nn_BFR_28089086116615).

v2 design (transposed-primary):
- Receiver axis i sharded (G=4096 -> 8 cores x 512). Edge matrices are
  host-transposed and gated in bf16: wT[j, i].
- Phase 1 computes h0^T = elu(W^T x) in a column-folded layout (4 row-groups
  of 32 features x 2048 cols) with 16 big N=512 bf16 matmuls, then 16 PE
  transposes ([128,128] blocks) rebuild the natural [gene-partition | 1|h]
  groups for the contraction lhsT. Per-core gene permutation (local genes
  first) keeps the local receiver slice at a fixed, core-independent offset.
- All small matmuls run in bf16 (1 cyc/row on PE vs 4 for fp32).
- Partition broadcasts (sdb row, rowsum, BN scale/shift) run on GpSimd
  instead of PE+DVE.
- BatchNorm is per-gene -> fully local, computed in row layout only.
- sigma^T is produced chunk-wise by ACT (sigmoid, per-partition bias =
  ssrc[j-chunk]), gated on DVE in bf16, contracted on PE with stationary
  [1|h] groups so the receiver rowsum lands in psum row 0.
- ACT work is batched by table set (sigmoid vs exp) to minimize the 1.3us
  activation table reloads.
- One AllGather of the normalized h (natural [1|h] groups) between blocks;
  scattered to SBUF with a single strided DMA.
"""
import sys
sys.path.insert(0, "/opt/trn_rl_repo")
import numpy as np
import ml_dtypes

import concourse.bass as bass
import concourse.bacc as bacc
import concourse.mybir as mybir
import concourse.tile as tile
from concourse import masks
from concourse.bass_utils import run_bass_kernel_spmd

NC = 8
B, G, NI, H, NO = 2, 4096, 8, 32, 32
GL = G // NC              # 512 local receivers per core
LCH = GL // 128           # 4 local chunks
NCH = G // 128            # 32 global j-chunks
W1 = H + 1                # group width: [1 | h]
ALPHA, BETA, BN_EPS = 0.005, 5e-5, 1e-5

F32 = mybir.dt.float32
F32R = mybir.dt.float32r
BF16 = mybir.dt.bfloat16
AF = mybir.ActivationFunctionType
ALU = mybir.AluOpType
XY = mybir.AxisListType.XY
AX = mybir.AxisListType.X

# pack_b column layout (bf16)
PK_WREP1 = 0                      # [128, NCH*W1] wrep1e (b_e1 in slot 0)
PK_WREP2 = NCH * W1               # [128, NCH*W1]
PK_BASE = 2 * NCH * W1            # 2112
PK_WAUG = PK_BASE                 # [9, 32]
PK_WE1D = PK_BASE + 32            # [32, 1]
PK_WE2D = PK_BASE + 33            # [32, 1]
PK_W8 = PK_BASE + 34              # 8 blocks of 32: Wn1a Wn1b Wm1a Wm1b Wn2a Wn2b Wm2a Wm2b
PK_W = PK_W8 + 8 * 32             # total width 2402
# pack_f column layout (fp32): 8 weight blocks, then bn_g/bn_b rows
PF_W8 = 0
PF_BNG = 256
PF_BNB = PF_BNG + GL
PF_ONER = PF_BNB + GL             # [1, GL] ones row
PF_ONEC = PF_ONER + GL            # [33, 1] ones column
PF_W = PF_ONEC + 1

_CACHE = {}


def build_program():
    nc = bacc.Bacc("TRN2", target_bir_lowering=False, debug=False,
                   enable_asserts=False, num_devices=NC)

    def din(name, shape, dt):
        return nc.dram_tensor(name, shape, dt, kind="ExternalInput").ap()

    xT_b = din("xT_b", [NI + 1, B * G], BF16)
    w1T = din("w1T", [G, GL], BF16)
    w2T = din("w2T", [G, GL], BF16)
    w2l = din("w2l", [GL, GL], BF16)
    pack_b = din("pack_b", [128, PK_W], BF16)
    pack_f = din("pack_f", [33, PF_W], F32R)

    out = nc.dram_tensor("out", [B * GL, NO], F32, kind="ExternalOutput").ap()
    out_r = out.rearrange("(b l p) f -> p b l f", b=B, l=LCH, p=128)

    with tile.TileContext(nc) as tc:
        with (
            tc.tile_pool(name="cp", bufs=1) as cp,
            tc.tile_pool(name="bp", bufs=1) as bp,
            tc.tile_pool(name="wp", bufs=1) as wp,
            tc.tile_pool(name="sp", bufs=2) as sp,
            tc.tile_pool(name="pp", bufs=1, space="PSUM") as pp,
            tc.tile_pool(name="dp", bufs=1, space="DRAM") as dp,
        ):
            # ---------------- input DMAs (small gating ones first) --------
            pack_sb = cp.tile([128, PK_W], BF16, name="pack_sb", tag="pack_sb")
            nc.sync.dma_start(pack_sb[:, PK_BASE:PK_W],
                              pack_b[:, PK_BASE:PK_W])
            xq = cp.tile([NI + 1, B * G], BF16, name="xq", tag="xq")
            for s in range(4):
                nc.sync.dma_start(xq[:, s * 2048:(s + 1) * 2048],
                                  xT_b[:, s * 2048:(s + 1) * 2048])
            pack_f_sb = cp.tile([33, PF_W], F32R, name="pack_f_sb",
                                tag="pack_f_sb")
            nc.sync.dma_start(pack_f_sb[:], pack_f[:])
            nc.sync.dma_start(pack_sb[:, 0:PK_BASE], pack_b[:, 0:PK_BASE])
            w1T_sb = bp.tile([128, NCH * GL], BF16, name="w1T_sb", tag="w1T_sb")
            w2T_sb = bp.tile([128, NCH * GL], BF16, name="w2T_sb", tag="w2T_sb")
            w1T_r = w1T.rearrange("(k p) i -> p k i", p=128)
            w2T_r = w2T.rearrange("(k p) i -> p k i", p=128)
            w2l_sb = bp.tile([128, LCH * GL], BF16, name="w2l_sb",
                             tag="w2l_sb")
            w2l_r = w2l.rearrange("(k p) i -> p k i", p=128)
            for kq in range(4):
                nc.sync.dma_start(
                    w1T_sb[:, kq * 8 * GL:(kq + 1) * 8 * GL],
                    w1T_r[:, kq * 8:(kq + 1) * 8])
            nc.sync.dma_start(w2l_sb[:], w2l_r[:])

            # views into the const pack
            wrep1_v = pack_sb[:, PK_WREP1:PK_WREP1 + NCH * W1].rearrange(
                "p (k e) -> p k e", e=W1)
            wrep2_v = pack_sb[:, PK_WREP2:PK_WREP2 + NCH * W1].rearrange(
                "p (k e) -> p k e", e=W1)
            W_aug = pack_sb[0:NI + 1, PK_WAUG:PK_WAUG + 32]
            We1_d = pack_sb[0:H, PK_WE1D:PK_WE1D + 1]
            We2_d = pack_sb[0:H, PK_WE2D:PK_WE2D + 1]

            def wblk(i, p):
                return pack_f_sb[0:p, PF_W8 + i * 32:PF_W8 + (i + 1) * 32]
            Wn1a, Wn1b = wblk(0, 33), wblk(1, 33)
            Wm1a, Wm1b = wblk(2, 33), wblk(3, 32)
            Wn2a, Wn2b = wblk(4, 33), wblk(5, 33)
            Wm2a, Wm2b = wblk(6, 33), wblk(7, 32)
            Wm1b_h = pack_sb[0:32, PK_W8 + 3 * 32:PK_W8 + 4 * 32]
            bn_g = pack_f_sb[0:1, PF_BNG:PF_BNG + GL].bitcast(F32)
            bn_b = pack_f_sb[0:1, PF_BNB:PF_BNB + GL].bitcast(F32)

            # ---------------- constants / identities ----------------------
            onesk_f = pack_f_sb[0:H, PF_ONEC:PF_ONEC + 1]
            oner_f = pack_f_sb[0:1, PF_ONER:PF_ONER + GL]
            id_bf = cp.tile([128, 128], BF16, name="id_bf", tag="id_bf")
            masks.make_identity(nc, id_bf[:])
            id_f = cp.tile([32, 32], F32, name="id_f", tag="id_f")
            masks.make_identity(nc, id_f[:])

            # ---------------- big resident tensors ------------------------
            h0T = bp.tile([128, 2048], BF16, name="h0T", tag="h0T")
            h0n = bp.tile([128, B * NCH * W1], BF16, name="h0n", tag="h0n")
            nc.vector.memset(h0n[:], 1.0)
            ghat = bp.tile([128, B * NCH * W1], BF16, name="ghat", tag="ghat")
            h1T = bp.tile([H, B * GL], F32R, name="h1T", tag="h1T")
            hbnT = bp.tile([H, B * GL], F32R, name="hbnT", tag="hbnT")
            hbn_b = bp.tile([H, B * GL], BF16, name="hbn_b", tag="hbn_b")
            GPW = B * LCH * W1 + 2 * B * LCH    # 280
            gather_in = dp.tile([128, GPW], BF16, name="gin", tag="gin")
            gather_out = dp.tile([NC * 128, GPW], BF16,
                                 addr_space="Shared", name="gout", tag="gout")

            hdTs, nodess = [], []
            for i in range(2):
                hdTx = wp.tile([W1, GL], F32R, name=f"hdT{i}", tag=f"hdT{i}")
                nc.vector.tensor_copy(hdTx[H:W1, :], oner_f)
                hdTs.append(hdTx)
                nodesx = wp.tile([W1, GL], F32R, name=f"nodes{i}",
                                 tag=f"nodes{i}")
                nc.vector.tensor_copy(nodesx[H:W1, :], oner_f)
                nodess.append(nodesx)

            def elu(z_psum, out_ap, p, f):
                # elu(z) = max(z, min(exp(z),1)-1); exp reads psum directly
                t = wp.tile([128, 512], BF16, name="elu_t", tag="elu_t",
                            bufs=3)[0:p, 0:f]
                nc.scalar.activation(t, z_psum, AF.Exp)
                nc.vector.tensor_scalar(t, t, 1.0, -1.0,
                                        op0=ALU.min, op1=ALU.add)
                nc.vector.tensor_tensor(out_ap, z_psum, t, op=ALU.max)

            # ---------------- phase 1: h0T fold + natural -----------------
            # fold[32a+f, 512s+j] = h0T[f, (4s+a)*512+j]
            for s in range(4):
                ps = pp.tile([128, 512], F32, name="ps_ph0", tag="ph0", bufs=2)
                for a in range(4):
                    q = 4 * s + a
                    nc.tensor.matmul(ps[32 * a:32 * a + 32, :], W_aug,
                                     xq[:, q * 512:(q + 1) * 512],
                                     start=True, stop=True,
                                     tile_position=(0, 32 * a))
                elu(ps[:], h0T[:, s * 512:(s + 1) * 512], 128, 512)
            # natural layout via PE transposes: group m = 16u + t1 + 4a
            h0n_g = h0n.rearrange("p (u a t e) -> p u a t e", u=4, a=4, t=4)
            for u in range(4):
                trp = pp.tile([128, 512], BF16, name="ps_tr", tag="tr", bufs=1)
                for t1 in range(4):
                    nc.tensor.transpose(
                        trp[:, t1 * 128:(t1 + 1) * 128],
                        h0T[:, (4 * u + t1) * 128:(4 * u + t1 + 1) * 128],
                        id_bf[:])
                trp_v = trp.rearrange("p (t a e) -> p t a e", t=4, a=4)
                nc.vector.tensor_copy(
                    h0n_g[:, u, :, :, 1:W1].transpose([0, 2, 1, 3]), trp_v)

            # local receiver features: h0l(b) = fold[0:32, 1024b : 1024b+512]
            def h0l(b):
                return h0T[0:H, 1024 * b:1024 * b + 512]

            h0n_v = h0n.rearrange("p (g e) -> p g e", e=W1)
            ghat_v = ghat.rearrange("p (g e) -> p g e", e=W1)

            # issue w2T loads behind w1T
            for kq in range(4):
                nc.sync.dma_start(
                    w2T_sb[:, kq * 8 * GL:(kq + 1) * 8 * GL],
                    w2T_r[:, kq * 8:(kq + 1) * 8])

            # ---------------- shared mp-block pieces ----------------------
            def ssrc_calc(blk, nat_v, wrep_v):
                ssrc = wp.tile([128, B * NCH], F32, name=f"ssrc{blk}",
                               tag=f"ssrc{blk}")
                for b in range(B):
                    ssx = wp.tile([128, NCH * W1], BF16, name="ssx", tag="ssx",
                                  bufs=2)
                    ssx_v = ssx.rearrange("p (k e) -> p k e", e=W1)
                    nc.vector.tensor_tensor(
                        ssx_v, nat_v[:, b * NCH:(b + 1) * NCH, :], wrep_v,
                        op=ALU.mult)
                    nc.vector.reduce_sum(ssrc[:, b * NCH:(b + 1) * NCH],
                                         ssx_v, axis=AX)
                return ssrc

            def sdb_calc(blk, We_d, hTl_of):
                sd_row = wp.tile([1, B * GL], BF16, name=f"sd{blk}",
                                 tag=f"sd{blk}")
                for b in range(B):
                    ps_d = pp.tile([1, 512], F32, name="ps_d", tag="sm",
                                   bufs=2)
                    nc.tensor.matmul(ps_d[:], We_d, hTl_of(b),
                                     start=True, stop=True)
                    nc.vector.tensor_copy(sd_row[:, b * GL:(b + 1) * GL],
                                          ps_d[:])
                sdb = wp.tile([128, B * GL], BF16, name=f"sdb{blk}",
                              tag=f"sdb{blk}")
                nc.gpsimd.partition_broadcast(sdb[:], sd_row[:])
                return sdb

            def contraction(b, wT_sb, sdb, ssrc, nat_v, acc=None,
                            cont=False):
                ps_acc = acc if acc is not None else pp.tile(
                    [W1, GL], F32, name="ps_acc", tag="acc", bufs=2)
                for qq in range(4):
                    sig = sp.tile([128, 8 * GL], BF16, name="sig", tag="sig",
                                  bufs=2)
                    for k8 in range(8):
                        k = qq * 8 + k8
                        nc.scalar.activation(
                            sig[:, k8 * GL:(k8 + 1) * GL],
                            sdb[:, b * GL:(b + 1) * GL], AF.Sigmoid,
                            bias=ssrc[:, b * NCH + k:b * NCH + k + 1])
                    for hh in range(2):
                        sl = slice(hh * 4 * GL, (hh + 1) * 4 * GL)
                        wsl = slice((qq * 8 + hh * 4) * GL,
                                    (qq * 8 + hh * 4 + 4) * GL)
                        nc.vector.tensor_tensor(sig[:, sl], sig[:, sl],
                                                wT_sb[:, wsl], op=ALU.mult)
                    for k8 in range(8):
                        k = qq * 8 + k8
                        nc.tensor.matmul(
                            ps_acc[:], nat_v[:, b * NCH + k, :],
                            sig[:, k8 * GL:(k8 + 1) * GL],
                            start=(k == 0 and not cont),
                            stop=(k == NCH - 1))
                return ps_acc

            def post_acc(b, ps_acc, hTl_b, hTl_mrg, Wna, Wnb, Wma, Wmb,
                         mrg_out, mrg_p):
                rfull = wp.tile([W1, GL], F32R, name="rfull", tag="rfull",
                                bufs=2)
                nc.vector.tensor_copy(rfull[:], ps_acc[:])
                rb = wp.tile([H, GL], F32R, name="rb", tag="rb", bufs=2)
                nc.gpsimd.partition_broadcast(rb[:], rfull[0:1, :])
                hdT = hdTs[b]
                nc.vector.tensor_tensor(hdT[0:H, :], hTl_b, rb.bitcast(F32),
                                        op=ALU.mult)
                ps_n = pp.tile([H, GL], F32, name="ps_n", tag="sm", bufs=2)
                nc.tensor.matmul(ps_n[:], Wna, rfull[:],
                                 start=True, stop=False)
                nc.tensor.matmul(ps_n[:], Wnb, hdT[:],
                                 start=False, stop=True)
                nodes = nodess[b]
                elu(ps_n[:], nodes[0:H, :], H, GL)
                ps_m = pp.tile([H, GL], F32, name="ps_m", tag="sm", bufs=2)
                nc.tensor.matmul(ps_m[:], Wma, nodes[:],
                                 start=True, stop=False)
                hT_m, Wmb_m = hTl_mrg
                nc.tensor.matmul(ps_m[:], Wmb_m, hT_m, start=False, stop=True)
                elu(ps_m[:], mrg_out, mrg_p, GL)

            # ---------------- block 1 -------------------------------------
            ssrc1 = ssrc_calc(1, h0n_v, wrep1_v)
            sdb1 = sdb_calc(1, We1_d, h0l)
            accs = [contraction(b, w1T_sb, sdb1, ssrc1, h0n_v)
                    for b in range(B)]
            for b in range(B):
                post_acc(b, accs[b], h0l(b), (h0l(b), Wm1b_h),
                         Wn1a, Wn1b, Wm1a, Wm1b,
                         h1T[:, b * GL:(b + 1) * GL], H)

            # ---------------- BatchNorm (row layout, fully local) ---------
            sqT = wp.tile([H, B * GL], F32R, name="sqT", tag="sqT")
            nc.vector.tensor_tensor(sqT[:], h1T.bitcast(F32), h1T.bitcast(F32),
                                    op=ALU.mult)
            ps_r0 = pp.tile([1, GL], F32, name="ps_r0", tag="sm", bufs=2)
            for b in range(B):
                nc.tensor.matmul(ps_r0[:], onesk_f[:],
                                 h1T[:, b * GL:(b + 1) * GL],
                                 start=(b == 0), stop=(b == B - 1))
            ps_r1 = pp.tile([1, GL], F32, name="ps_r1", tag="sm", bufs=2)
            for b in range(B):
                nc.tensor.matmul(ps_r1[:], onesk_f[:],
                                 sqT[:, b * GL:(b + 1) * GL],
                                 start=(b == 0), stop=(b == B - 1))
            rowb = wp.tile([1, 4 * GL], F32, name="rowb", tag="rowb")
            mu_r, var_r = rowb[:, 0:GL], rowb[:, GL:2 * GL]
            scl_r, shf_r = rowb[:, 2 * GL:3 * GL], rowb[:, 3 * GL:4 * GL]
            nc.vector.tensor_scalar_mul(mu_r, ps_r0[:], 1.0 / (B * NO))
            nc.vector.tensor_scalar_mul(var_r, ps_r1[:], 1.0 / (B * NO))
            nc.vector.tensor_tensor(scl_r, mu_r, mu_r, op=ALU.mult)
            nc.vector.tensor_tensor(var_r, var_r, scl_r, op=ALU.subtract)
            nc.vector.tensor_scalar_add(scl_r, var_r, BN_EPS)
            nc.scalar.activation(scl_r, scl_r, AF.Ln)
            nc.scalar.activation(scl_r, scl_r, AF.Exp, scale=-0.5)
            nc.vector.tensor_tensor(scl_r, scl_r, bn_g, op=ALU.mult)
            nc.vector.tensor_tensor(shf_r, mu_r, scl_r, op=ALU.mult)
            nc.vector.tensor_tensor(shf_r, bn_b, shf_r, op=ALU.subtract)
            ssb = wp.tile([H, 2 * GL], F32, name="ssb", tag="ssb")
            nc.gpsimd.partition_broadcast(ssb[:], rowb[:, 2 * GL:4 * GL])
            for b in range(B):
                sl = slice(b * GL, (b + 1) * GL)
                nc.vector.tensor_tensor(hbnT[:, sl],
                                        h1T[:, sl].bitcast(F32), ssb[:, 0:GL],
                                        op=ALU.mult)
                nc.vector.tensor_tensor(hbnT[:, sl], hbnT[:, sl].bitcast(F32),
                                        ssb[:, GL:2 * GL], op=ALU.add)
            nc.vector.tensor_copy(hbn_b[:], hbnT.bitcast(F32))

            # block-2 sdb can go before the gather (local only)
            sdb2 = sdb_calc(2, We2_d,
                            lambda b: hbn_b[:, b * GL:(b + 1) * GL])

            # ---------------- gather of normalized h + ssrc2 scalars ------
            # payload: [ (b,l) (1|h) groups : 264 | s2 scalars as f32 : 16 ]
            hbn_n = wp.tile([128, GPW], BF16, name="hbn_n", tag="hbn_n")
            nc.vector.memset(hbn_n[:], 1.0)
            hbn_nv = hbn_n[:, 0:B * LCH * W1].rearrange("p (g e) -> p g e",
                                                        e=W1)
            s2fs = []
            for b in range(B):
                trh = pp.tile([128, 512], BF16, name="ps_trh", tag="tr",
                              bufs=1)
                for l in range(LCH):
                    j = b * LCH + l
                    nc.tensor.transpose(trh[:, l * 32:(l + 1) * 32],
                                        hbn_b[:, j * 128:(j + 1) * 128],
                                        id_bf[0:H, 0:H])
                trh_v = trh[:, 0:LCH * H].rearrange("p (g e) -> p g e", e=H)
                nc.vector.tensor_copy(
                    hbn_nv[:, b * LCH:(b + 1) * LCH, 1:W1], trh_v)
                ssx2 = wp.tile([128, LCH * W1], BF16, name="ssx2", tag="ssx2",
                               bufs=2)
                ssx2_v = ssx2.rearrange("p (k e) -> p k e", e=W1)
                nc.vector.tensor_tensor(ssx2_v,
                                        hbn_nv[:, b * LCH:(b + 1) * LCH, :],
                                        wrep2_v[:, 0:LCH, :], op=ALU.mult)
                s2f = wp.tile([128, LCH], F32, name="s2f", tag="s2f", bufs=2)
                nc.vector.reduce_sum(s2f[:], ssx2_v, axis=AX)
                s2fs.append(s2f)
                dst = hbn_n[:, B * LCH * W1 + 2 * b * LCH:
                            B * LCH * W1 + 2 * (b + 1) * LCH].bitcast(F32)
                nc.vector.tensor_copy(dst, s2f[:])
            nc.sync.dma_start(gather_in[:], hbn_n[:])
            nc.gpsimd.collective_compute(
                "AllGather", ALU.bypass, replica_groups=[list(range(NC))],
                ins=[gather_in.opt()], outs=[gather_out.opt()])

            # -------- local-first block 2 (own rows of w2T are zeroed) -----
            accs2 = []
            for b in range(B):
                ps_acc = pp.tile([W1, GL], F32, name="ps_acc2", tag="acc",
                                 bufs=2)
                sig = sp.tile([128, LCH * GL], BF16, name="sigl", tag="sig",
                              bufs=2)
                for l in range(LCH):
                    nc.scalar.activation(
                        sig[:, l * GL:(l + 1) * GL],
                        sdb2[:, b * GL:(b + 1) * GL], AF.Sigmoid,
                        bias=s2fs[b][:, l:l + 1])
                nc.vector.tensor_tensor(sig[:], sig[:], w2l_sb[:],
                                        op=ALU.mult)
                for l in range(LCH):
                    nc.tensor.matmul(ps_acc[:],
                                     hbn_nv[:, b * LCH + l, :],
                                     sig[:, l * GL:(l + 1) * GL],
                                     start=(l == 0), stop=False)
                accs2.append(ps_acc)

            ssrc2 = wp.tile([128, B * NCH], F32, name="ssrc2", tag="ssrc2")
            for b in range(B):
                gr = gather_out[:, b * LCH * W1:(b + 1) * LCH * W1].rearrange(
                    "(c p) x -> p c x", p=128)
                nc.sync.dma_start(
                    ghat[:, b * NCH * W1:(b + 1) * NCH * W1].rearrange(
                        "p (c x) -> p c x", c=NC), gr)
                sr = gather_out[:, B * LCH * W1 + 2 * b * LCH:
                                B * LCH * W1 + 2 * (b + 1) * LCH].bitcast(
                    F32).rearrange("(c p) l -> p c l", p=128)
                nc.sync.dma_start(
                    ssrc2[:, b * NCH:(b + 1) * NCH].rearrange(
                        "p (c l) -> p c l", c=NC), sr)

            # ---------------- block 2 -------------------------------------
            out2T = wp.tile([H, B * GL], F32, name="out2T", tag="out2T")
            for b in range(B):
                contraction(b, w2T_sb, sdb2, ssrc2, ghat_v,
                            acc=accs2[b], cont=True)
            for b in range(B):
                sl2 = slice(b * GL, (b + 1) * GL)
                post_acc(b, accs2[b], hbnT[:, sl2].bitcast(F32),
                         (hbnT[:, sl2], Wm2b),
                         Wn2a, Wn2b, Wm2a, Wm2b,
                         out2T[:, sl2], H)

            # transpose to natural and store
            tro = pp.tile([128, B * LCH * NO], F32, name="ps_tro", tag="trf",
                          bufs=1)
            for j in range(B * LCH):
                nc.tensor.transpose(tro[:, j * 32:(j + 1) * 32],
                                    out2T[:, j * 128:(j + 1) * 128],
                                    id_f[:])
            out_n = wp.tile([128, B * LCH * NO], F32, name="out_n",
                            tag="out_n")
            nc.vector.tensor_copy(out_n[:], tro[:])
            nc.sync.dma_start(
                out_r, out_n.rearrange("p (b l f) -> p b l f", b=B, l=LCH))

    nc.compile()
    return nc


def _prep_inputs(x, edges1, edges2, W_infer, b_infer, W_e1, b_e1, W_e2, b_e2,
                 W_n1, b_n1, W_n2, b_n2, W_m1, b_m1, W_m2, b_m2,
                 bn_gamma, bn_beta):
    f32 = np.float32
    bf16 = ml_dtypes.bfloat16
    x = np.asarray(x, f32)
    w1 = (ALPHA + (1.0 - ALPHA) * np.asarray(edges1, f32)).astype(bf16)
    w2 = (BETA + (1.0 - BETA) * np.asarray(edges2, f32)).astype(bf16)

    def wrepe(W_e, b_e):
        row = np.concatenate([[np.asarray(b_e, f32)[0]],
                              np.asarray(W_e, f32)[:H, 0]])
        return np.tile(row, (128, NCH)).astype(bf16)

    z = np.zeros((1, NO), f32)

    def stk(Wpart, brow):
        return np.concatenate([np.asarray(Wpart, f32), brow], 0)

    pack = np.zeros((128, PK_W), f32)
    pack[:, PK_WREP1:PK_WREP1 + NCH * W1] = wrepe(W_e1, b_e1)
    pack[:, PK_WREP2:PK_WREP2 + NCH * W1] = wrepe(W_e2, b_e2)
    pack[0:NI + 1, PK_WAUG:PK_WAUG + 32] = np.concatenate(
        [np.asarray(W_infer, f32), np.asarray(b_infer, f32)[None, :]], 0)
    pack[0:H, PK_WE1D] = np.asarray(W_e1, f32)[H:, 0]
    pack[0:H, PK_WE2D] = np.asarray(W_e2, f32)[H:, 0]
    blks = [
        (np.concatenate([z, np.asarray(W_n1, f32)[:H]], 0), 33),
        (stk(np.asarray(W_n1, f32)[H:], np.asarray(b_n1, f32)[None, :]), 33),
        (stk(np.asarray(W_m1, f32)[:H], np.asarray(b_m1, f32)[None, :]), 33),
        (np.asarray(W_m1, f32)[H:], 32),
        (np.concatenate([z, np.asarray(W_n2, f32)[:H]], 0), 33),
        (stk(np.asarray(W_n2, f32)[H:], np.asarray(b_n2, f32)[None, :]), 33),
        (stk(np.asarray(W_m2, f32)[:H], np.asarray(b_m2, f32)[None, :]), 33),
        (np.asarray(W_m2, f32)[H:], 32),
    ]
    packf = np.zeros((33, PF_W), f32)
    for i, (w, p) in enumerate(blks):
        pack[0:p, PK_W8 + i * 32:PK_W8 + (i + 1) * 32] = w
        packf[0:p, PF_W8 + i * 32:PF_W8 + (i + 1) * 32] = w
    packf[0, PF_ONER:PF_ONER + GL] = 1.0
    packf[:, PF_ONEC] = 1.0
    pack = pack.astype(bf16)

    in_maps = []
    for c in range(NC):
        sl = slice(c * GL, (c + 1) * GL)
        # per-core gene permutation for block 1: local genes first
        perm = np.concatenate([np.arange(c * GL, (c + 1) * GL),
                               np.arange(0, c * GL),
                               np.arange((c + 1) * GL, G)])
        xp = x[:, perm, :]                       # [B, G, NI] permuted
        xT = xp.transpose(2, 0, 1).reshape(NI, B * G)
        xT_b = np.concatenate([xT, np.ones((1, B * G), f32)], 0).astype(bf16)
        pf = packf.copy()
        pf[0, PF_BNG:PF_BNG + GL] = np.asarray(bn_gamma, f32)[sl]
        pf[0, PF_BNB:PF_BNB + GL] = np.asarray(bn_beta, f32)[sl]
        w2z = np.asarray(w2)[sl, :].T.copy()     # [j, i]; zero own senders
        w2z[c * GL:(c + 1) * GL, :] = 0
        w2z = np.ascontiguousarray(w2z)
        m = dict(
            xT_b=xT_b,
            w1T=np.ascontiguousarray(
                np.asarray(w1)[sl, :][:, perm].T),
            w2T=w2z,
            w2l=np.ascontiguousarray(np.asarray(w2)[sl, sl].T),
            pack_b=pack,
            pack_f=pf,
        )
        in_maps.append(m)
    return in_maps


def kernel(**inputs):
    if "nc" not in _CACHE:
        _CACHE["nc"] = build_program()
    nc = _CACHE["nc"]
    in_maps = _prep_inputs(**inputs)
    res = run_bass_kernel_spmd(nc, in_maps, list(range(NC)))
    parts = [res.results[c]["out"].reshape(B, GL, NO) for c in range(NC)]
    return np.concatenate(parts, axis=1).astype(np.float32)
